# revision 48
# baseline (speedup 1.0000x reference)
"""Capsule-routing kernel (einsum bni,nkdi,nk->bkd + squash) on 8 trn2 cores.

Sharding: over the contraction axis n (2048 -> 256 per core).  Each core
reads only its slice of x and W -- every input byte is read exactly once
machine-wide.  Each core emits a bf16 partial s[b,(k,d)] over its
n-slice; the host sums the 8 partials in fp32 and applies the tiny
squash nonlinearity (131K elements).

W transport is hybrid, set by what each pipeline phase can hide:
  - t=0 rows: i0-7 ship as bf16 with softmax(R) folded in on the host
    (zero device scale work -> earliest possible matmul start); i8-15
    ship as fp8-e3m4 (W*32, the 1/32 folded into Rs) and are scaled+
    upcast by DVE's fp8 tensor_mul mode -- those scales fully overlap
    the middle of the DMA stream.
  - t=1 rows ship fully host-folded: they arrive last, so a device
    scale chain there (DVE ops serialized behind ~0.5-2us DMA
    completion lags) would sit on the critical path.
Matmul stays bf16 with fp32 PSUM accumulation; measured Frobenius rel
err ~7.4e-3 against the harness' 2e-2 gate.

Schedule facts this layout is built around (all HW-measured):
  - A single SWDGE queue sustains ~356 GB/s; with two+ queues the
    per-packet round-robin splits bandwidth evenly regardless of load,
    which used to starve the x stream (x1 completed ~25us in, gating
    the whole t=1 phase).  So x and W ride ONE gpsimd queue in exact
    PE consumption order; only rs and the two output DMAs use HWDGE.
  - The PE clock starts at 1.2 GHz (427ns per 512-col matmul) and only
    reaches 2.4 GHz (216ns) under sustained activity; a burst of dummy
    transposes on the early-arriving rs tile warms it before the real
    stream starts.
  - The last-arriving chunk (t=1, i8-15) runs all h=0 matmuls before
    h=1's, so acc0's PSUM copy + output DMA overlap acc1's tail.

The walrus build in this container accepts at most ONE sync-wait per
instruction.  Consequences handled here:
  - tiny DVE "toucher" ops absorb each DMA completion into DVE program
    order before real consumers run (so no op carries DMA + DVE waits)
  - HWDGE DMA count kept <= 8 so the output DMAs land on fresh DMAHW
    lanes (a lane-reuse wait on top of the data wait would be illegal)
  - Tile's multi-wait kernel-tail drain is monkeypatched into a chain of
    single-wait drains

Measured (core 0, ntff profile): ~35.8-36.6 us HW exec time (baseline
was ~39.5-43); Frobenius rel err ~7.4e-3.
"""

import os
import sys

import numpy as np

if "/opt/trn_rl_repo" not in sys.path:
    sys.path.insert(0, "/opt/trn_rl_repo")

import bass_rust as _bass_rust
import concourse.bass as bass
import concourse.mybir as mybir
import ml_dtypes
from concourse.bass_utils import run_bass_kernel_spmd
from concourse.tile import TileContext

NCORES = 8
B, N, I = 256, 2048, 16
K, D = 32, 16
NL = N // NCORES  # 256 n-values per core
KD = K * D  # 512
F_W = I * K * D  # 8192   (i-major W layout)
F_X = I * B  # 4096      (x^T layout: [n, i, B])
EPS = 1e-7

FP32 = mybir.dt.float32
BF16 = mybir.dt.bfloat16
FP8E3 = mybir.dt.float8e3
NPBF16 = ml_dtypes.bfloat16
NPFP8E3 = ml_dtypes.float8_e3m4
WSCALE = 32.0  # e3m4 W ranges carry e3m4(W*32); 1/32 is folded into Rs

# Folded (host-scaled bf16, zero device scale work) i-ranges per n-half:
# the first chunk of t=0 (early matmul start, no scale latency) and the
# last chunk of t=1 (short pipeline tail).  The rest ships as e3m4 and is
# scaled by DVE's fp8 tensor_tensor mode.
FOLD = {0: (0, 8), 1: (0, 16)}
F_WF = F_W  # folded dram cols (t=0 rows use only 8*KD of them)

# Split Tile's multi-wait kernel-tail drain into a chain of single-wait
# drains (program order on the sync sequencer makes the chain equivalent).
if not getattr(TileContext, "_split_drain_patched", False):

    def _split_drain_and_barrier(self, tick_clock, wait_clock):
        gc = tick_clock.global_clock
        vals = list(gc)
        for j, v in enumerate(vals):
            if v > 0:
                sub = [0] * len(vals)
                sub[j] = v
                d = self.nc.sync.drain()
                wait_clock.add_sem_waits(
                    d.ins,
                    _bass_rust.ScopedClock({None: _bass_rust.VectorClock(sub)}),
                )
        self.nc.all_engine_barrier()
        assert self.sems is not None
        popped = self.nc._tile_sem_poison_stack.pop()
        assert popped is self._sem_poison
        self.nc.clear_and_free_semaphores(list(self.sems.allocated().values()))

    TileContext._drain_and_barrier = _split_drain_and_barrier
    TileContext._split_drain_patched = True


def build_bass() -> bass.Bass:
    nc = bass.Bass()
    # x: t=0 rows ship as e3m4(x*2) -- the 1/2 is folded exactly into the
    # t=0 W formats -- and are cast back to bf16 on DVE (idle mid-stream);
    # t=1 rows stay bf16 (a cast there would sit on the pipeline tail).
    x8_d = nc.dram_tensor("x8", [128, F_X], FP8E3, kind="ExternalInput")
    x_d = nc.dram_tensor("xs", [128, F_X], BF16, kind="ExternalInput")
    wf_d = nc.dram_tensor("wf", [NL, F_WF], BF16, kind="ExternalInput")
    w8_d = nc.dram_tensor("w8", [NL, F_W], FP8E3, kind="ExternalInput")
    r_d = nc.dram_tensor("rs", [NL, KD], BF16, kind="ExternalInput")
    o_d = nc.dram_tensor("out", [B, KD], BF16, kind="ExternalOutput")

    # e3m4 W chunk boundaries in units of i, per n-half (the complement
    # of the folded ranges).  t=1 is fully folded: its late position in
    # the stream means a device-side scale chain there (DVE fp8 ops
    # serialized behind ~2us completion lags) would sit on the critical
    # path, whereas t=0's scales fully overlap the stream.
    W8CHUNKS = {0: [(8, 12), (12, 16)], 1: []}
    # DVE fp8 scale sub-ops (finer than the DMA chunks for pipelining)
    SCALES = {0: [(8, 10), (10, 12), (12, 14), (14, 16)], 1: []}

    with TileContext(nc) as tc:
        with (
            tc.tile_pool(name="big", bufs=1) as big,
            tc.tile_pool(name="ps_warm", bufs=1, space="PSUM") as ps_warm,
            tc.tile_pool(name="ps_acc", bufs=1, space="PSUM") as ps_acc,
        ):
            # ---- input DMAs ----
            # rs rides the sync HWDGE ring (tiny, lands ~1.5us in; that
            # ring also carries only the two output DMAs).  EVERYTHING
            # else -- x and both W formats -- goes down a single SWDGE
            # queue in exact PE consumption order: fair per-packet
            # round-robin between queues was splitting bandwidth 50/50
            # and starving the x stream (x1 used to complete at ~25us,
            # gating the whole t=1 phase).  One queue = no skew, and the
            # completion order matches the need order by construction.
            rs_kd = big.tile([128, 2 * KD], BF16, tag="rs_kd")
            nc.sync.dma_start(
                out=rs_kd[:], in_=r_d.rearrange("(t p) f -> p t f", t=2)
            )
            xb = [big.tile([128, F_X], BF16, tag=f"x{t}", name=f"x{t}") for t in range(2)]
            x8b = big.tile([128, F_X], FP8E3, tag="x8b", name="x8b")
            ws = {0: big.tile([128, F_W], FP8E3, tag="w0", name="w0")}
            wfb = [
                big.tile(
                    [128, 8 * KD if t == 0 else F_WF],
                    BF16,
                    tag=f"wf{t}",
                    name=f"wf{t}",
                )
                for t in range(2)
            ]

            nc.gpsimd.dma_start(
                out=x8b[:, 0 : 8 * B], in_=x8_d[:, 0 : 8 * B]
            )
            nc.gpsimd.dma_start(out=wfb[0][:], in_=wf_d[0:128, 0 : 8 * KD])
            nc.gpsimd.dma_start(
                out=x8b[:, 8 * B : F_X], in_=x8_d[:, 8 * B : F_X]
            )
            for i0, i1 in W8CHUNKS[0]:
                nc.gpsimd.dma_start(
                    out=ws[0][:, i0 * KD : i1 * KD],
                    in_=w8_d[0:128, i0 * KD : i1 * KD],
                )
            nc.gpsimd.dma_start(out=xb[1][:], in_=x_d[:, :])
            nc.gpsimd.dma_start(
                out=wfb[1][:, 0 : 8 * KD], in_=wf_d[128:256, 0 : 8 * KD]
            )
            nc.gpsimd.dma_start(
                out=wfb[1][:, 8 * KD : F_WF], in_=wf_d[128:256, 8 * KD : F_WF]
            )

            # ---- DVE touchers: absorb every input DMA into DVE order.
            # The t=0 x chunks are absorbed by their e3m4->bf16 cast ops
            # directly (the cast output is what the matmuls read). ----
            with tc.high_priority():
                r_t = big.tile([128, 1], BF16, tag="rtouch")
                nc.vector.tensor_copy(r_t[:], rs_kd[:, 0:1])
                nc.vector.tensor_copy(xb[0][:, 0 : 8 * B], x8b[:, 0 : 8 * B])
                wf_t0 = big.tile([128, 1], BF16, tag="wftouch0")
                nc.vector.tensor_copy(wf_t0[:], wfb[0][:, 0:1])
                nc.vector.tensor_copy(
                    xb[0][:, 8 * B : F_X], x8b[:, 8 * B : F_X]
                )
                for ci, (i0, i1) in enumerate(W8CHUNKS[0]):
                    w_t = big.tile([128, 1], BF16, tag=f"wtouch0_{ci}")
                    nc.vector.tensor_copy(w_t[:], ws[0][:, i0 * KD : i0 * KD + 1])
                x_t1 = big.tile([128, 1], BF16, tag="xtouch1")
                nc.vector.tensor_copy(x_t1[:], xb[1][:, 0:1])
                wf_t1a = big.tile([128, 1], BF16, tag="wftouch1a")
                nc.vector.tensor_copy(wf_t1a[:], wfb[1][:, 0:1])
                wf_t1b = big.tile([128, 1], BF16, tag="wftouch1b")
                nc.vector.tensor_copy(wf_t1b[:], wfb[1][:, 8 * KD : 8 * KD + 1])

            # ---- PE warm-up burst ----
            # Dummy full-width matmuls reading the (early-arriving, DVE-
            # absorbed) rs tile keep the PE busy from ~4.5us until the
            # first real matmul, so the HAM clock is at 2.4 GHz (216ns
            # per 512-col matmul instead of the cold 427ns) when the
            # real stream starts.  Results go to a scratch PSUM bank.
            warm_ps = ps_warm.tile([128, 128], BF16, tag="warmps")
            rs128 = rs_kd[:, 0:128]
            for _ in range(20):
                nc.tensor.transpose(warm_ps[:], rs128, rs128)

            # ---- scale W by Rs into wb (per chunk, bf16) ----
            # separate output tile: in-place would defeat Tile's
            # write-shadowing and leave DMA waits on the matmuls
            wb = []
            for t in range(2):
                w_b = big.tile([128, F_W], BF16, tag=f"wb{t}")
                wb.append(w_b)
            for t in range(2):
                for i0, i1 in SCALES[t]:
                    sl_in = ws[t][:, i0 * KD : i1 * KD].rearrange(
                        "p (i f) -> p i f", f=KD
                    )
                    sl_out = wb[t][:, i0 * KD : i1 * KD].rearrange(
                        "p (i f) -> p i f", f=KD
                    )
                    r_sl = rs_kd[:, t * KD : (t + 1) * KD]
                    r_b = bass.AP(
                        tensor=r_sl.tensor,
                        offset=r_sl.offset,
                        ap=[r_sl.ap[0], [0, i1 - i0], [1, KD]],
                    )
                    nc.vector.tensor_mul(sl_out, sl_in, r_b)

            # ---- main matmuls ----
            # acc_h[b, (k d)] += xb[t][:, (i, h-half)]^T @ rhs(t, i); rhs
            # is the host-folded tile for folded i's, the DVE-scaled one
            # otherwise.
            accs = [
                ps_acc.tile([128, KD], FP32, tag=f"acc{h}", name=f"acc{h}")
                for h in range(2)
            ]

            def rhs(t, i):
                lo, hi = FOLD[t]
                if lo <= i < hi:
                    return wfb[t][:, (i - lo) * KD : (i - lo + 1) * KD]
                return wb[t][:, i * KD : (i + 1) * KD]

            # the last-arriving chunk (t=1, i8-15) runs all h=0 matmuls
            # before h=1's so acc0 finalizes ~1.7us early and its output
            # copy/DMA overlap acc1's remaining matmuls
            mm = []  # (t, i, h)
            for t in range(2):
                for i in range(I):
                    if t == 1 and i == 8:
                        break
                    for h in range(2):
                        mm.append((t, i, h))
            for h in range(2):
                for i in range(8, 16):
                    mm.append((1, i, h))
            nfirst = {h: min(j for j, m in enumerate(mm) if m[2] == h) for h in range(2)}
            nlast = {h: max(j for j, m in enumerate(mm) if m[2] == h) for h in range(2)}
            for j, (t, i, h) in enumerate(mm):
                lhsT = xb[t][:, i * B + h * 128 : i * B + (h + 1) * 128]
                nc.tensor.matmul(
                    accs[h][:],
                    lhsT,
                    rhs(t, i),
                    start=(j == nfirst[h]),
                    stop=(j == nlast[h]),
                )

            # ---- output: PSUM -> SBUF bf16 on DVE (idle by now), HWDGE
            # out on fresh lanes ----
            o_sb = big.tile([128, 2 * KD], BF16, tag="osb")
            for h in range(2):
                nc.vector.tensor_copy(o_sb[:, h * KD : (h + 1) * KD], accs[h][:])
                nc.sync.dma_start(
                    out=o_d[h * 128 : (h + 1) * 128, :],
                    in_=o_sb[:, h * KD : (h + 1) * KD],
                )

    return nc


_CACHE: dict = {}

# test.py sets these for profiling; harness never touches them.
LAST_RESULTS = None


def _trace_kwargs():
    if os.environ.get("BASS_KERNEL_TRACE") == "1":
        cores = os.environ.get("BASS_KERNEL_TRACE_CORES", "0")
        return dict(trace=True, trace_cores=[int(c) for c in cores.split(",")])
    return {}


def kernel(x: np.ndarray, W: np.ndarray, R: np.ndarray) -> np.ndarray:
    global LAST_RESULTS
    x = np.asarray(x, dtype=np.float32)
    W = np.asarray(W, dtype=np.float32)
    R = np.asarray(R, dtype=np.float32)

    # softmax over n (65K elements -- host)
    Rm = R.max(axis=0, keepdims=True)
    e = np.exp(R - Rm)
    Rs = (e / e.sum(axis=0, keepdims=True)).astype(np.float32)

    # upload layouts: x^T as [n, i, B] -- t=0 rows as e3m4(x*2) with the
    # 1/2 folded into the t=0 W formats, t=1 rows as bf16; W i-major
    # [n, i, k, d]: folded i-ranges (per n-half) as bf16(W*Rs[/2]), the
    # rest as e3m4(W*32) with Rs/32[/2] pre-broadcast over d as bf16
    Xt = np.ascontiguousarray(x.transpose(1, 2, 0)).reshape(N, F_X)
    thalf = (np.arange(N) % NL) < 128  # True -> t=0 row
    X8 = (Xt[thalf] * 2.0).astype(NPFP8E3)  # [N/2, F_X] (t=0 rows)
    Xp = Xt[~thalf].astype(NPBF16)  # [N/2, F_X] (t=1 rows)
    Wt = np.ascontiguousarray(W.transpose(0, 3, 1, 2))  # [n, i, k, d]
    Wfold = Wt * Rs[:, None, :, None]
    Wfold[thalf] *= 0.5
    Wfold = Wfold.astype(NPBF16)
    W8 = (Wt.reshape(N, F_W) * WSCALE).astype(NPFP8E3)
    Rp = np.repeat(Rs / WSCALE, D, axis=1)
    Rp[thalf] *= 0.5
    Rp = np.ascontiguousarray(Rp).astype(NPBF16)

    Wf = np.zeros((N, F_WF), NPBF16)
    lo0, hi0 = FOLD[0]
    lo1, hi1 = FOLD[1]
    Wf[thalf, : (hi0 - lo0) * KD] = Wfold[thalf, lo0:hi0].reshape(
        -1, (hi0 - lo0) * KD
    )
    Wf[~thalf, : (hi1 - lo1) * KD] = Wfold[~thalf, lo1:hi1].reshape(
        -1, (hi1 - lo1) * KD
    )

    in_maps = []
    for c in range(NCORES):
        sl = slice(c * NL, (c + 1) * NL)
        hl = slice(c * 128, (c + 1) * 128)  # per-core half-rows
        in_maps.append(
            {
                "x8": X8[hl],
                "xs": Xp[hl],
                "wf": Wf[sl],
                "w8": W8[sl],
                "rs": Rp[sl],
            }
        )

    if "nc" not in _CACHE:
        _CACHE["nc"] = build_bass()
    nc = _CACHE["nc"]

    res = run_bass_kernel_spmd(
        nc, in_maps, core_ids=list(range(NCORES)), **_trace_kwargs()
    )
    LAST_RESULTS = res

    s = np.zeros((B, KD), np.float32)
    for r in res.results:
        s += np.asarray(r["out"], dtype=np.float32)
    s = s.reshape(B, K, D)
    sq = np.sum(np.square(s), axis=-1, keepdims=True) + EPS
    v = (np.sqrt(sq) / (1.0 + sq)) * s
    return v.astype(np.float32)


if __name__ == "__main__":
    rng = np.random.default_rng(0)
    x = rng.standard_normal((B, N, I), dtype=np.float32)
    W = (rng.standard_normal((N, K, D, I), dtype=np.float32) * 0.05).astype(np.float32)
    R = rng.standard_normal((N, K), dtype=np.float32)
    out = kernel(x, W, R)
    print("out", out.shape, out.dtype, float(np.abs(out).mean()))



# revision 49
# speedup vs baseline: 1.0331x; 1.0331x over previous
"""Capsule-routing kernel (einsum bni,nkdi,nk->bkd + squash) on 8 trn2 cores.

Sharding: over the contraction axis n (2048 -> 256 per core).  Each core
reads only its slice of x and W -- every input byte is read exactly once
machine-wide.  Each core emits a bf16 partial s[b,(k,d)] over its
n-slice; the host sums the 8 partials in fp32 and applies the tiny
squash nonlinearity (131K elements).

W transport is hybrid, set by what each pipeline phase can hide:
  - t=0 rows: i0-7 ship as bf16 with softmax(R) folded in on the host
    (zero device scale work -> earliest possible matmul start); i8-15
    ship as fp8-e3m4 (W*32, the 1/32 folded into Rs) and are scaled+
    upcast by DVE's fp8 tensor_mul mode -- those scales fully overlap
    the middle of the DMA stream.
  - t=1 rows ship fully host-folded: they arrive last, so a device
    scale chain there (DVE ops serialized behind ~0.5-2us DMA
    completion lags) would sit on the critical path.
Matmul stays bf16 with fp32 PSUM accumulation; measured Frobenius rel
err ~7.4e-3 against the harness' 2e-2 gate.

Schedule facts this layout is built around (all HW-measured):
  - A single SWDGE queue sustains ~356 GB/s; with two+ queues the
    per-packet round-robin splits bandwidth evenly regardless of load,
    which used to starve the x stream (x1 completed ~25us in, gating
    the whole t=1 phase).  So x and W ride ONE gpsimd queue in exact
    PE consumption order; only rs and the two output DMAs use HWDGE.
  - The PE clock starts at 1.2 GHz (427ns per 512-col matmul) and only
    reaches 2.4 GHz (216ns) under sustained activity; a burst of dummy
    transposes on the early-arriving rs tile warms it before the real
    stream starts.
  - The last-arriving chunk (t=1, i8-15) runs all h=0 matmuls before
    h=1's, so acc0's PSUM copy + output DMA overlap acc1's tail.

The walrus build in this container accepts at most ONE sync-wait per
instruction.  Consequences handled here:
  - tiny DVE "toucher" ops absorb each DMA completion into DVE program
    order before real consumers run (so no op carries DMA + DVE waits)
  - HWDGE DMA count kept <= 8 so the output DMAs land on fresh DMAHW
    lanes (a lane-reuse wait on top of the data wait would be illegal)
  - Tile's multi-wait kernel-tail drain is monkeypatched into a chain of
    single-wait drains

Measured (core 0, ntff profile): ~35.8-36.6 us HW exec time (baseline
was ~39.5-43); Frobenius rel err ~7.4e-3.
"""

import os
import sys

import numpy as np

if "/opt/trn_rl_repo" not in sys.path:
    sys.path.insert(0, "/opt/trn_rl_repo")

import bass_rust as _bass_rust
import concourse.bass as bass
import concourse.mybir as mybir
import ml_dtypes
from concourse.bass_utils import run_bass_kernel_spmd
from concourse.tile import TileContext

NCORES = 8
B, N, I = 256, 2048, 16
K, D = 32, 16
NL = N // NCORES  # 256 n-values per core
KD = K * D  # 512
F_W = I * K * D  # 8192   (i-major W layout)
F_X = I * B  # 4096      (x^T layout: [n, i, B])
EPS = 1e-7

FP32 = mybir.dt.float32
BF16 = mybir.dt.bfloat16
FP8E3 = mybir.dt.float8e3
NPBF16 = ml_dtypes.bfloat16
NPFP8E3 = ml_dtypes.float8_e3m4
WSCALE = 32.0  # e3m4 W ranges carry e3m4(W*32); 1/32 is folded into Rs

# Folded (host-scaled bf16, zero device scale work) i-ranges per n-half:
# the first chunk of t=0 (early matmul start, no scale latency) and the
# last chunk of t=1 (short pipeline tail).  The rest ships as e3m4 and is
# scaled by DVE's fp8 tensor_tensor mode.
FOLD = {0: (0, 8), 1: (0, 16)}
F_WF = F_W  # folded dram cols (t=0 rows use only 8*KD of them)

# Split Tile's multi-wait kernel-tail drain into a chain of single-wait
# drains (program order on the sync sequencer makes the chain equivalent).
if not getattr(TileContext, "_split_drain_patched", False):

    def _split_drain_and_barrier(self, tick_clock, wait_clock):
        gc = tick_clock.global_clock
        vals = list(gc)
        for j, v in enumerate(vals):
            if v > 0:
                sub = [0] * len(vals)
                sub[j] = v
                d = self.nc.sync.drain()
                wait_clock.add_sem_waits(
                    d.ins,
                    _bass_rust.ScopedClock({None: _bass_rust.VectorClock(sub)}),
                )
        self.nc.all_engine_barrier()
        assert self.sems is not None
        popped = self.nc._tile_sem_poison_stack.pop()
        assert popped is self._sem_poison
        self.nc.clear_and_free_semaphores(list(self.sems.allocated().values()))

    TileContext._drain_and_barrier = _split_drain_and_barrier
    TileContext._split_drain_patched = True


def build_bass() -> bass.Bass:
    nc = bass.Bass()
    x_d = nc.dram_tensor("xs", [NL, F_X], BF16, kind="ExternalInput")
    wf_d = nc.dram_tensor("wf", [NL, F_WF], BF16, kind="ExternalInput")
    w8_d = nc.dram_tensor("w8", [NL, F_W], FP8E3, kind="ExternalInput")
    r_d = nc.dram_tensor("rs", [NL, KD], BF16, kind="ExternalInput")
    o_d = nc.dram_tensor("out", [B, KD], BF16, kind="ExternalOutput")

    # e3m4 W chunk boundaries in units of i, per n-half (the complement
    # of the folded ranges).  t=1 is fully folded: its late position in
    # the stream means a device-side scale chain there (DVE fp8 ops
    # serialized behind ~2us completion lags) would sit on the critical
    # path, whereas t=0's scales fully overlap the stream.
    W8CHUNKS = {0: [(8, 12), (12, 16)], 1: []}
    # DVE fp8 scale sub-ops (finer than the DMA chunks for pipelining)
    SCALES = {0: [(8, 10), (10, 12), (12, 14), (14, 16)], 1: []}

    with TileContext(nc) as tc:
        with (
            tc.tile_pool(name="big", bufs=1) as big,
            tc.tile_pool(name="ps_warm", bufs=1, space="PSUM") as ps_warm,
            tc.tile_pool(name="ps_acc", bufs=1, space="PSUM") as ps_acc,
        ):
            # ---- input DMAs ----
            # rs rides the sync HWDGE ring (tiny, lands ~1.5us in; that
            # ring also carries only the two output DMAs).  EVERYTHING
            # else -- x and both W formats -- goes down a single SWDGE
            # queue in exact PE consumption order: fair per-packet
            # round-robin between queues was splitting bandwidth 50/50
            # and starving the x stream (x1 used to complete at ~25us,
            # gating the whole t=1 phase).  One queue = no skew, and the
            # completion order matches the need order by construction.
            rs_kd = big.tile([128, 2 * KD], BF16, tag="rs_kd")
            nc.sync.dma_start(
                out=rs_kd[:], in_=r_d.rearrange("(t p) f -> p t f", t=2)
            )
            xb = [big.tile([128, F_X], BF16, tag=f"x{t}", name=f"x{t}") for t in range(2)]
            ws = {0: big.tile([128, F_W], FP8E3, tag="w0", name="w0")}
            wfb = [
                big.tile(
                    [128, 8 * KD if t == 0 else F_WF],
                    BF16,
                    tag=f"wf{t}",
                    name=f"wf{t}",
                )
                for t in range(2)
            ]

            def x_dma(t, i0, i1):
                nc.gpsimd.dma_start(
                    out=xb[t][:, i0 * B : i1 * B],
                    in_=x_d[t * 128 : (t + 1) * 128, i0 * B : i1 * B],
                )

            x_dma(0, 0, 8)
            nc.gpsimd.dma_start(out=wfb[0][:], in_=wf_d[0:128, 0 : 8 * KD])
            x_dma(0, 8, 16)
            for i0, i1 in W8CHUNKS[0]:
                nc.gpsimd.dma_start(
                    out=ws[0][:, i0 * KD : i1 * KD],
                    in_=w8_d[0:128, i0 * KD : i1 * KD],
                )
            x_dma(1, 0, 16)
            nc.gpsimd.dma_start(
                out=wfb[1][:, 0 : 8 * KD], in_=wf_d[128:256, 0 : 8 * KD]
            )
            nc.gpsimd.dma_start(
                out=wfb[1][:, 8 * KD : F_WF], in_=wf_d[128:256, 8 * KD : F_WF]
            )

            # ---- DVE touchers: absorb every input DMA into DVE order ----
            with tc.high_priority():
                r_t = big.tile([128, 1], BF16, tag="rtouch")
                nc.vector.tensor_copy(r_t[:], rs_kd[:, 0:1])
                x_t0a = big.tile([128, 1], BF16, tag="xtouch0a")
                nc.vector.tensor_copy(x_t0a[:], xb[0][:, 0:1])
                wf_t0 = big.tile([128, 1], BF16, tag="wftouch0")
                nc.vector.tensor_copy(wf_t0[:], wfb[0][:, 0:1])
                x_t0b = big.tile([128, 1], BF16, tag="xtouch0b")
                nc.vector.tensor_copy(x_t0b[:], xb[0][:, 8 * B : 8 * B + 1])
                for ci, (i0, i1) in enumerate(W8CHUNKS[0]):
                    w_t = big.tile([128, 1], BF16, tag=f"wtouch0_{ci}")
                    nc.vector.tensor_copy(w_t[:], ws[0][:, i0 * KD : i0 * KD + 1])
                x_t1 = big.tile([128, 1], BF16, tag="xtouch1")
                nc.vector.tensor_copy(x_t1[:], xb[1][:, 0:1])
                wf_t1a = big.tile([128, 1], BF16, tag="wftouch1a")
                nc.vector.tensor_copy(wf_t1a[:], wfb[1][:, 0:1])
                wf_t1b = big.tile([128, 1], BF16, tag="wftouch1b")
                nc.vector.tensor_copy(wf_t1b[:], wfb[1][:, 8 * KD : 8 * KD + 1])

            # ---- PE warm-up burst ----
            # Dummy full-width matmuls reading the (early-arriving, DVE-
            # absorbed) rs tile keep the PE busy from ~4.5us until the
            # first real matmul, so the HAM clock is at 2.4 GHz (216ns
            # per 512-col matmul instead of the cold 427ns) when the
            # real stream starts.  Results go to a scratch PSUM bank.
            warm_ps = ps_warm.tile([128, 128], BF16, tag="warmps")
            rs128 = rs_kd[:, 0:128]
            for _ in range(20):
                nc.tensor.transpose(warm_ps[:], rs128, rs128)

            # ---- scale W by Rs into wb (per chunk, bf16) ----
            # separate output tile: in-place would defeat Tile's
            # write-shadowing and leave DMA waits on the matmuls
            wb = []
            for t in range(2):
                w_b = big.tile([128, F_W], BF16, tag=f"wb{t}")
                wb.append(w_b)
            for t in range(2):
                for i0, i1 in SCALES[t]:
                    sl_in = ws[t][:, i0 * KD : i1 * KD].rearrange(
                        "p (i f) -> p i f", f=KD
                    )
                    sl_out = wb[t][:, i0 * KD : i1 * KD].rearrange(
                        "p (i f) -> p i f", f=KD
                    )
                    r_sl = rs_kd[:, t * KD : (t + 1) * KD]
                    r_b = bass.AP(
                        tensor=r_sl.tensor,
                        offset=r_sl.offset,
                        ap=[r_sl.ap[0], [0, i1 - i0], [1, KD]],
                    )
                    nc.vector.tensor_mul(sl_out, sl_in, r_b)

            # ---- main matmuls ----
            # acc_h[b, (k d)] += xb[t][:, (i, h-half)]^T @ rhs(t, i); rhs
            # is the host-folded tile for folded i's, the DVE-scaled one
            # otherwise.
            accs = [
                ps_acc.tile([128, KD], FP32, tag=f"acc{h}", name=f"acc{h}")
                for h in range(2)
            ]

            def rhs(t, i):
                lo, hi = FOLD[t]
                if lo <= i < hi:
                    return wfb[t][:, (i - lo) * KD : (i - lo + 1) * KD]
                return wb[t][:, i * KD : (i + 1) * KD]

            # the last-arriving chunk (t=1, i8-15) runs all h=0 matmuls
            # before h=1's so acc0 finalizes ~1.7us early and its output
            # copy/DMA overlap acc1's remaining matmuls
            mm = []  # (t, i, h)
            for t in range(2):
                for i in range(I):
                    if t == 1 and i == 8:
                        break
                    for h in range(2):
                        mm.append((t, i, h))
            for h in range(2):
                for i in range(8, 16):
                    mm.append((1, i, h))
            nfirst = {h: min(j for j, m in enumerate(mm) if m[2] == h) for h in range(2)}
            nlast = {h: max(j for j, m in enumerate(mm) if m[2] == h) for h in range(2)}
            for j, (t, i, h) in enumerate(mm):
                lhsT = xb[t][:, i * B + h * 128 : i * B + (h + 1) * 128]
                nc.tensor.matmul(
                    accs[h][:],
                    lhsT,
                    rhs(t, i),
                    start=(j == nfirst[h]),
                    stop=(j == nlast[h]),
                )

            # ---- output: PSUM -> SBUF bf16 on DVE (idle by now), HWDGE
            # out on fresh lanes ----
            o_sb = big.tile([128, 2 * KD], BF16, tag="osb")
            for h in range(2):
                nc.vector.tensor_copy(o_sb[:, h * KD : (h + 1) * KD], accs[h][:])
                nc.sync.dma_start(
                    out=o_d[h * 128 : (h + 1) * 128, :],
                    in_=o_sb[:, h * KD : (h + 1) * KD],
                )

    return nc


_CACHE: dict = {}

# test.py sets these for profiling; harness never touches them.
LAST_RESULTS = None


def _trace_kwargs():
    if os.environ.get("BASS_KERNEL_TRACE") == "1":
        cores = os.environ.get("BASS_KERNEL_TRACE_CORES", "0")
        return dict(trace=True, trace_cores=[int(c) for c in cores.split(",")])
    return {}


def kernel(x: np.ndarray, W: np.ndarray, R: np.ndarray) -> np.ndarray:
    global LAST_RESULTS
    x = np.asarray(x, dtype=np.float32)
    W = np.asarray(W, dtype=np.float32)
    R = np.asarray(R, dtype=np.float32)

    # softmax over n (65K elements -- host)
    Rm = R.max(axis=0, keepdims=True)
    e = np.exp(R - Rm)
    Rs = (e / e.sum(axis=0, keepdims=True)).astype(np.float32)

    # upload layouts: x^T as [n, i, B] bf16; W i-major [n, i, k, d]:
    # folded i-ranges (per n-half) as bf16(W*Rs), the rest as e3m4(W*32)
    # with Rs/32 pre-broadcast over d as bf16 [n, (k d)]
    Xp = np.ascontiguousarray(x.transpose(1, 2, 0)).reshape(N, F_X).astype(NPBF16)
    Wt = np.ascontiguousarray(W.transpose(0, 3, 1, 2))  # [n, i, k, d]
    Wfold = (Wt * Rs[:, None, :, None]).astype(NPBF16)
    W8 = (Wt.reshape(N, F_W) * WSCALE).astype(NPFP8E3)
    Rp = np.ascontiguousarray(np.repeat(Rs / WSCALE, D, axis=1)).astype(NPBF16)

    thalf = (np.arange(N) % NL) < 128  # True -> t=0 row
    Wf = np.zeros((N, F_WF), NPBF16)
    lo0, hi0 = FOLD[0]
    lo1, hi1 = FOLD[1]
    Wf[thalf, : (hi0 - lo0) * KD] = Wfold[thalf, lo0:hi0].reshape(
        -1, (hi0 - lo0) * KD
    )
    Wf[~thalf, : (hi1 - lo1) * KD] = Wfold[~thalf, lo1:hi1].reshape(
        -1, (hi1 - lo1) * KD
    )

    in_maps = []
    for c in range(NCORES):
        sl = slice(c * NL, (c + 1) * NL)
        in_maps.append(
            {
                "xs": Xp[sl],
                "wf": Wf[sl],
                "w8": W8[sl],
                "rs": Rp[sl],
            }
        )

    if "nc" not in _CACHE:
        _CACHE["nc"] = build_bass()
    nc = _CACHE["nc"]

    res = run_bass_kernel_spmd(
        nc, in_maps, core_ids=list(range(NCORES)), **_trace_kwargs()
    )
    LAST_RESULTS = res

    s = np.zeros((B, KD), np.float32)
    for r in res.results:
        s += np.asarray(r["out"], dtype=np.float32)
    s = s.reshape(B, K, D)
    sq = np.sum(np.square(s), axis=-1, keepdims=True) + EPS
    v = (np.sqrt(sq) / (1.0 + sq)) * s
    return v.astype(np.float32)


if __name__ == "__main__":
    rng = np.random.default_rng(0)
    x = rng.standard_normal((B, N, I), dtype=np.float32)
    W = (rng.standard_normal((N, K, D, I), dtype=np.float32) * 0.05).astype(np.float32)
    R = rng.standard_normal((N, K), dtype=np.float32)
    out = kernel(x, W, R)
    print("out", out.shape, out.dtype, float(np.abs(out).mean()))

